# revision 12
# baseline (speedup 1.0000x reference)
"""Trainium2 Bass kernel for nn_Compositional: sigmoid(sum(er*ea*eb, -1)).

  ea = x @ W_ent.T   [N, D]
  eb = y @ W_ent.T   [N, D]
  er = r @ W_rel.T   [N, D]
  out = sigmoid(sum_d er*ea*eb)  [N, 1]

Sharding: data-parallel over N across 8 cores (512 rows each), W_ent/W_rel
replicated.

Per-core plan (v2 — bf16 datapath):
  - x/y/W_ent are cast fp32->bf16 *during* the HBM->SBUF DMA (SWDGE cast
    on gpsimd), halving DMA-engine occupancy vs fp32 loads.
  - All ea/eb matmuls run in bf16 (fp32 PSUM accumulation): same 1
    cycle/row PE rate as fp32r, but transposes drop from 1.5 to 1.0
    cycles/row.
  - All transposes run on PE (tensor.transpose via identity).  A DMA-xbar
    (dma_start_transpose) offload path exists behind XBAR_Y_GROUPS but is
    disabled: it produced wrong results on HW and simmed slower due to
    scheduler sem-lane serialization.
  - PE instruction stream is software-pipelined: chunk c's matmuls are
    issued after chunk c+1's transposes so PE never stalls on the
    PSUM->SBUF copy latency.  Loads are issued 3 groups ahead of compute
    in program order so the SWDGE FIFO never head-of-line blocks.
  - r/W_rel/er stay fp32r (cheap, keeps precision).  Epilogue
    (er*ea*eb, partition-reduce via ones-matmul, sigmoid) is pipelined
    per d-half to shorten the tail.
"""
import os

import numpy as np

# Full-problem constants (hardcoded; kernel.py must be self-contained).
N, E, R, D = 4096, 16384, 512, 256
NCORES = 8
NC_N = N // NCORES      # 512 rows per core
EG = 1024               # e-columns per x/y group
NG = E // EG            # 16 groups
CPG = EG // 128         # 8 contraction chunks per group
NCHUNK = E // 128       # 128 contraction chunks
DH = D // 128           # 2 d-halves

# Groups whose *y* transposes go through the DMA xbar instead of PE
# (x always transposes on PE so the PE stream stays dense).
XBAR_Y_GROUPS = frozenset()

_CACHE = {}


def _build():
    import concourse.mybir as mybir
    import concourse.tile as tile
    from concourse import bacc
    from concourse.masks import make_identity

    F32 = mybir.dt.float32
    F32R = mybir.dt.float32r
    BF16 = mybir.dt.bfloat16
    MUL = mybir.AluOpType.mult

    nc = bacc.Bacc("TRN2", target_bir_lowering=False)

    x_dram = nc.dram_tensor("x", [NC_N, E], F32, kind="ExternalInput")
    y_dram = nc.dram_tensor("y", [NC_N, E], F32, kind="ExternalInput")
    r_dram = nc.dram_tensor("r", [NC_N, R], F32, kind="ExternalInput")
    went_dram = nc.dram_tensor("W_ent", [D, E], F32, kind="ExternalInput")
    wrel_dram = nc.dram_tensor("W_rel", [D, R], F32, kind="ExternalInput")
    out_dram = nc.dram_tensor("out", [NC_N, 1], F32, kind="ExternalOutput")

    with tile.TileContext(nc) as tc:
        with (
            tc.tile_pool(name="const", bufs=1) as cpool,
            tc.tile_pool(name="stream", bufs=1) as pool,
            tc.tile_pool(name="psum", bufs=1, space="PSUM") as psum,
        ):
            # ---- constants ----
            ident = cpool.tile([128, 128], F32)
            make_identity(nc, ident[:])
            identb = cpool.tile([128, 128], BF16)
            nc.vector.tensor_copy(identb[:], ident[:])
            identr = cpool.tile([128, 128], F32R)
            nc.vector.tensor_copy(identr[:], ident[:])
            ones_f = cpool.tile([128, 1], F32)
            nc.gpsimd.memset(ones_f[:], 1.0)
            ones_r = cpool.tile([128, 1], F32R)
            nc.vector.tensor_copy(ones_r[:], ones_f[:])

            # ---- resident tensors ----
            went_t = cpool.tile([128, NCHUNK, D], BF16)       # [e_in, chunk, d]
            wrel_t = cpool.tile([128, R // 128, D], F32R)     # [p_in, pchunk, d]
            ert_sb = cpool.tile([128, DH, NC_N], F32)         # [d_in, dh, n]

            # ---- PSUM accumulators (persist through main loop) ----
            ea_ps = [
                psum.tile([128, NC_N], F32, tag=f"ea{dh}", bufs=1, name=f"ea{dh}")
                for dh in range(DH)
            ]
            eb_ps = [
                psum.tile([128, NC_N], F32, tag=f"eb{dh}", bufs=1, name=f"eb{dh}")
                for dh in range(DH)
            ]

            went_r = went_dram.rearrange("(dh p) e -> p dh e", p=128)

            def w_load(g, split=1):
                w_nat = pool.tile([128, DH, EG], BF16, tag="w_nat", bufs=3,
                                  name="w_nat")
                sw = EG // split
                for s_ in range(split):
                    nc.gpsimd.dma_start(
                        w_nat[:, :, s_ * sw : (s_ + 1) * sw],
                        went_r[:, :, g * EG + s_ * sw : g * EG + (s_ + 1) * sw],
                    )
                return w_nat

            def w_transpose(g, w_nat):
                """PE-transpose W_ent e-cols [g*EG, (g+1)*EG)."""
                for dh in range(DH):
                    for h in range(CPG // 4):
                        wt_ps = psum.tile(
                            [128, 512], BF16, tag="work", bufs=4, name="wt_ps"
                        )
                        for q in range(4):
                            c = h * 4 + q
                            nc.tensor.transpose(
                                wt_ps[:, q * 128 : (q + 1) * 128],
                                w_nat[:, dh, c * 128 : (c + 1) * 128],
                                identb[:],
                            )
                        nc.vector.tensor_copy(
                            went_t[
                                :,
                                g * CPG + 4 * h : g * CPG + 4 * h + 4,
                                dh * 128 : (dh + 1) * 128,
                            ],
                            wt_ps[:].rearrange("p (j e) -> p j e", j=4),
                        )

            def xy_load(g, split=1, y_first=False):
                """Cast-load x/y e-cols for group g -> bf16 [128, 4, EG]."""
                x_nat = pool.tile([128, 4, EG], BF16, tag="x_nat", bufs=4,
                                  name="x_nat")
                y_nat = pool.tile([128, 4, EG], BF16, tag="y_nat", bufs=5,
                                  name="y_nat")
                sw = EG // split
                def load(nat, dram):
                    for s_ in range(split):
                        nc.gpsimd.dma_start(
                            nat[:, :, s_ * sw : (s_ + 1) * sw],
                            dram[:, g * EG + s_ * sw : g * EG + (s_ + 1) * sw]
                            .rearrange("(j p) e -> p j e", p=128),
                        )
                if y_first:
                    load(y_nat, y_dram); load(x_nat, x_dram)
                else:
                    load(x_nat, x_dram); load(y_nat, y_dram)
                return x_nat, y_nat

            # Software-pipelined matmul issue: queue of (chunk, rhs_x, rhs_y)

            pending = []

            def flush_matmuls(keep=0):
                while len(pending) > keep:
                    chunk, rx, ry = pending.pop(0)
                    last = chunk == NCHUNK - 1
                    for dh in range(DH):
                        nc.tensor.matmul(
                            ea_ps[dh][:],
                            went_t[:, chunk, dh * 128 : (dh + 1) * 128],
                            rx,
                            start=(chunk == 0),
                            stop=last,
                        )
                        nc.tensor.matmul(
                            eb_ps[dh][:],
                            went_t[:, chunk, dh * 128 : (dh + 1) * 128],
                            ry,
                            start=(chunk == 0),
                            stop=last,
                        )

            def xy_group_pe(g, x_nat, y_nat):
                """PE-transpose path for group g."""
                for c4 in range(CPG):
                    chunk = g * CPG + c4
                    xt_ps = psum.tile(
                        [128, NC_N], BF16, tag="work", bufs=4, name="xt_ps"
                    )
                    for j in range(4):
                        nc.tensor.transpose(
                            xt_ps[:, j * 128 : (j + 1) * 128],
                            x_nat[:, j, c4 * 128 : (c4 + 1) * 128],
                            identb[:],
                        )
                    xt_sb = pool.tile(
                        [128, NC_N], BF16, tag="xt_sb", bufs=3, name="xt_sb"
                    )
                    nc.scalar.copy(xt_sb[:], xt_ps[:])
                    yt_ps = psum.tile(
                        [128, NC_N], BF16, tag="work", bufs=4, name="yt_ps"
                    )
                    for j in range(4):
                        nc.tensor.transpose(
                            yt_ps[:, j * 128 : (j + 1) * 128],
                            y_nat[:, j, c4 * 128 : (c4 + 1) * 128],
                            identb[:],
                        )
                    yt_sb = pool.tile(
                        [128, NC_N], BF16, tag="yt_sb", bufs=3, name="yt_sb"
                    )
                    nc.vector.tensor_copy(yt_sb[:], yt_ps[:])
                    pending.append((chunk, xt_sb[:], yt_sb[:]))
                    flush_matmuls(keep=1)

            def xy_group_ybar(g, x_nat, yt_dma):
                """x on PE, y already transposed by the DMA xbar at load time."""
                for c4 in range(CPG):
                    chunk = g * CPG + c4
                    xt_ps = psum.tile(
                        [128, NC_N], BF16, tag="work", bufs=4, name="xt_ps"
                    )
                    for j in range(4):
                        nc.tensor.transpose(
                            xt_ps[:, j * 128 : (j + 1) * 128],
                            x_nat[:, j, c4 * 128 : (c4 + 1) * 128],
                            identb[:],
                        )
                    xt_sb = pool.tile(
                        [128, NC_N], BF16, tag="xt_sb", bufs=3, name="xt_sb"
                    )
                    nc.scalar.copy(xt_sb[:], xt_ps[:])
                    pending.append((chunk, xt_sb[:], yt_dma[:, :, c4, :]))
                    flush_matmuls(keep=1)

            rel_tiles = {}

            def rel_loads():
                wr_nats = []
                for dh in range(DH):
                    wr_nat = pool.tile(
                        [128, 512], F32R, tag="wr_nat", bufs=2, name="wr_nat"
                    )
                    nc.sync.dma_start(
                        wr_nat[:],
                        wrel_dram[dh * 128 : (dh + 1) * 128, :].bitcast(F32R),
                    )
                    wr_nats.append(wr_nat)
                r_nats = []
                for pc in range(R // 128):
                    r_nat = pool.tile(
                        [128, 4, 128], F32R, tag="r_nat", bufs=4, name="r_nat"
                    )
                    nc.sync.dma_start(
                        r_nat[:],
                        r_dram[:, pc * 128 : (pc + 1) * 128]
                        .rearrange("(j p) e -> p j e", p=128)
                        .bitcast(F32R),
                    )
                    r_nats.append(r_nat)
                rel_tiles["wr"] = wr_nats
                rel_tiles["r"] = r_nats

            def rel_phase():
                """W_rel -> W_relT, r -> rT, er matmuls, erT -> SBUF (f32r)."""
                for dh in range(DH):
                    wr_nat = rel_tiles["wr"][dh]
                    wrt_ps = psum.tile(
                        [128, 512], F32R, tag="work", bufs=4, name="wrt_ps"
                    )
                    for j in range(4):
                        nc.tensor.transpose(
                            wrt_ps[:, j * 128 : (j + 1) * 128],
                            wr_nat[:, j * 128 : (j + 1) * 128],
                            identr[:],
                        )
                    nc.vector.tensor_copy(
                        wrel_t[:, :, dh * 128 : (dh + 1) * 128],
                        wrt_ps[:].rearrange("p (j e) -> p j e", j=4),
                    )

                er_ps = [
                    psum.tile([128, NC_N], F32, tag="work", bufs=4, name=f"er{dh}")
                    for dh in range(DH)
                ]
                for pc in range(R // 128):
                    r_nat = rel_tiles["r"][pc]
                    rt_ps = psum.tile(
                        [128, NC_N], F32R, tag="work", bufs=4, name="rt_ps"
                    )
                    for j in range(4):
                        nc.tensor.transpose(
                            rt_ps[:, j * 128 : (j + 1) * 128], r_nat[:, j], identr[:]
                        )
                    rt_sb = pool.tile(
                        [128, NC_N], F32R, tag="xt_sb", bufs=3, name="rt_sb"
                    )
                    nc.scalar.copy(rt_sb[:], rt_ps[:])
                    for dh in range(DH):
                        nc.tensor.matmul(
                            er_ps[dh][:],
                            wrel_t[:, pc, dh * 128 : (dh + 1) * 128],
                            rt_sb[:],
                            start=(pc == 0),
                            stop=(pc == R // 128 - 1),
                        )
                for dh in range(DH):
                    nc.scalar.copy(ert_sb[:, dh, :], er_ps[dh][:])

            # ---- main schedule: loads run 2 groups ahead of compute ----
            LOOKAHEAD = 3
            wn = {}
            xyn = {}

            def issue_loads(g):
                if g < NG:
                    wn[g] = w_load(g, split=(2 if g == 0 else 1))
                    x_nat, y_nat = xy_load(
                        g, split=(4 if g == 0 else 1),
                        y_first=(g in XBAR_Y_GROUPS),
                    )
                    yt_dma = None
                    if g in XBAR_Y_GROUPS:
                        yt_dma = pool.tile(
                            [128, 4, CPG, 128], BF16, tag="yt_dma", bufs=2,
                            name="yt_dma",
                        )
                        nc.sync.dma_start_transpose(
                            yt_dma[:].rearrange("p j c m -> p (j c) m"),
                            y_nat[:].rearrange("p j e -> p (j e)"),
                        )
                    xyn[g] = (x_nat, y_nat, yt_dma)

            rel_loads()
            for g in range(LOOKAHEAD + 1):
                issue_loads(g)
            for g in range(NG):
                w_transpose(g, wn.pop(g))
                xn, yn, ytd = xyn.pop(g)
                if g in XBAR_Y_GROUPS:
                    xy_group_ybar(g, xn, ytd)
                else:
                    xy_group_pe(g, xn, yn)
                if g == 0:
                    rel_phase()
                issue_loads(g + LOOKAHEAD + 1)
            flush_matmuls()

            # ---- epilogue (pipelined per d-half) ----
            score_ps = psum.tile([1, NC_N], F32, tag="work", bufs=4, name="score_ps")
            for dh in range(DH):
                t_sb = pool.tile([128, NC_N], F32, tag="t_sb", bufs=2, name="t_sb")
                nc.vector.tensor_tensor(t_sb[:], ea_ps[dh][:], ert_sb[:, dh, :], MUL)
                p_sb = pool.tile([128, NC_N], F32R, tag="p_sb", bufs=2, name="p_sb")
                nc.vector.tensor_tensor(p_sb[:], eb_ps[dh][:], t_sb[:], MUL)
                nc.tensor.matmul(
                    score_ps[:],
                    ones_r[:],
                    p_sb[:],
                    start=(dh == 0),
                    stop=(dh == DH - 1),
                )
            sig_sb = pool.tile([1, NC_N], F32, name="sig_sb")
            nc.scalar.activation(
                sig_sb[:], score_ps[:], mybir.ActivationFunctionType.Sigmoid
            )
            nc.sync.dma_start(out_dram[:].rearrange("n o -> o n"), sig_sb[:])

    nc.compile()
    return nc


def _get_nc():
    if "nc" not in _CACHE:
        _CACHE["nc"] = _build()
    return _CACHE["nc"]


def kernel(x, y, r, W_ent, W_rel):
    from concourse.bass_utils import run_bass_kernel_spmd

    x = np.ascontiguousarray(np.asarray(x, dtype=np.float32))
    y = np.ascontiguousarray(np.asarray(y, dtype=np.float32))
    r = np.ascontiguousarray(np.asarray(r, dtype=np.float32))
    W_ent = np.ascontiguousarray(np.asarray(W_ent, dtype=np.float32))
    W_rel = np.ascontiguousarray(np.asarray(W_rel, dtype=np.float32))

    nc = _get_nc()
    in_maps = [
        {
            "x": x[c * NC_N : (c + 1) * NC_N],
            "y": y[c * NC_N : (c + 1) * NC_N],
            "r": r[c * NC_N : (c + 1) * NC_N],
            "W_ent": W_ent,
            "W_rel": W_rel,
        }
        for c in range(NCORES)
    ]
    trace = bool(int(os.environ.get("KERNEL_TRACE", "0")))
    res = run_bass_kernel_spmd(
        nc, in_maps, core_ids=list(range(NCORES)), trace=trace
    )
    _CACHE["last_result"] = res
    out = np.concatenate([res.results[c]["out"] for c in range(NCORES)], axis=0)
    return out
